# revision 21
# baseline (speedup 1.0000x reference)
"""Trainium2 Bass kernel for nn_MlpMixer_18966575579742.

Complex-valued per-frequency (j) MLP:
  o1r = gelu(xr@w1[0] - xi@w1[1] + b1[0]);  o1i = gelu(xi@w1[0] + xr@w1[1] + b1[1])
  o2r = o1r@w2[0] - o1i@w2[1] + b2[0];      o2i = o1i@w2[0] + o1i@w2[1] + b2[1]
  (note: o2i intentionally uses o1i with BOTH w2[0] and w2[1], as in the source)

Sharding over 8 cores: 2 j-halves (13 each) x 4 batch-quarters (B=32 -> 512 rows).

Per-core dataflow, all matmuls in bf16 (1 PE pass @ ~216ns for N=512 vs fp32's
2 passes @ ~432ns -- 4x less PE time; fp32 PSUM accumulation keeps the
contraction exact, tolerance is 2e-2):
  - host pre-transposes x shards to [j, k, c, rows] bf16 so both real and
    imag arrive in one DMA per j, already in streaming layout
  - L1 uses the direct 4-matmul complex product accumulated in PSUM
    (p1r = xr@w1[0] + xi@(-w1[1]), p1i = xi@w1[0] + xr@w1[1]); vs Gauss
    3-mult this costs +1 bf16 pass per h-chunk but eliminates the 3 DVE
    combine ops per chunk that would otherwise make Vector the bottleneck
  - exact-erf GELU + per-partition b1 bias runs on ScalarE reading PSUM
    directly (ScalarE has the fast PSUM port), writing bf16 o1 to SBUF
  - L2 (w2 stationary, o1 moving): o2T [k'=128, rows] PSUM accumulated via
    w2[0], -w2[1] (real) and w2[0]+w2[1] (imag) -- 3 passes per h-chunk
  - DVE drains PSUM with fused per-partition b2 bias, writing bf16
  - output stays transposed [j, c, k', rows] bf16; host does the final
    transpose + complex interleave
  - biases are DMA'd in clean row-major staging tiles and PE-transposed once
  - DMA issue split across queues: x/out on sync, weights/bias on gpsimd
    (ScalarE issues no DMA -- it needs its full time for GELU)
  - software pipeline: L1(j+1) is issued to the PE before L2(j), so the PE
    never stalls waiting for GELU(j) to finish
"""

import sys

if "/opt/trn_rl_repo" not in sys.path:
    sys.path.insert(0, "/opt/trn_rl_repo")

import numpy as np
import ml_dtypes

BF16 = ml_dtypes.bfloat16

B, I, J, K, F = 128, 16, 26, 128, 4
H = K * F  # 512
NJG = 2  # j groups
NRG = 4  # row (batch) groups
JL = J // NJG  # 13 j per core
BL = B // NRG  # 32 batches per core
ROWS = BL * I  # 512 rows per core
NHC = H // 128  # 4 h-chunks

_cache = {}


def _build_nc():
    from contextlib import ExitStack

    import concourse.mybir as mybir
    import concourse.tile as tile
    from concourse import bacc
    from concourse.masks import make_identity

    f32 = mybir.dt.float32
    bf16 = mybir.dt.bfloat16
    nc = bacc.Bacc(None)

    # x arrives pre-transposed: [j, k, c, rows] (c = real/imag), bf16
    xp = nc.declare_dram_parameter("xp", [JL, K, 2, ROWS], bf16, isOutput=False)
    w1 = nc.declare_dram_parameter("w1", [JL, K, 2, H], bf16, isOutput=False)
    # w2 pre-shuffled to [j, p, c, hc, k'] where h = hc*128 + p
    w2 = nc.declare_dram_parameter("w2", [JL, K, 2, NHC, K], bf16, isOutput=False)
    b1 = nc.declare_dram_parameter("b1", [2, JL, H], f32, isOutput=False)
    b2 = nc.declare_dram_parameter("b2", [2, JL, K], f32, isOutput=False)
    # transposed output: [j, c, k', rows] bf16; host fixes layout
    out = nc.declare_dram_parameter("out", [JL, 2, K, ROWS], bf16, isOutput=True)

    GELU = mybir.ActivationFunctionType.Gelu

    with tile.TileContext(nc) as tc, ExitStack() as ctx:
        const = ctx.enter_context(tc.tile_pool(name="const", bufs=1))
        wp = ctx.enter_context(tc.tile_pool(name="wp", bufs=2))
        wnp = ctx.enter_context(tc.tile_pool(name="wnp", bufs=2))
        xtp = ctx.enter_context(tc.tile_pool(name="xtp", bufs=2))
        o1p = ctx.enter_context(tc.tile_pool(name="o1p", bufs=2))
        outp = ctx.enter_context(tc.tile_pool(name="outp", bufs=4))
        ps1 = ctx.enter_context(tc.tile_pool(name="ps1", bufs=4, space="PSUM"))
        ps2 = ctx.enter_context(tc.tile_pool(name="ps2", bufs=4, space="PSUM"))

        identity = const.tile([128, 128], f32)
        make_identity(nc, identity)

        # PE warm-up: the HAM clock gate keeps the PE at 1.2 GHz until it has
        # been busy for a full ~3.4us activity window. Burn that window on
        # dummy zero matmuls while the first DMAs are still in flight, so
        # every real matmul runs at 2.4 GHz. The memset goes on gpsimd, whose
        # const-init work runs during the NEFF preamble window.
        wz = const.tile([128, 256], bf16)
        nc.vector.memset(wz, 0.0)
        wps = ps1.tile([128, 256], f32, tag="p1")
        # enough to bridge from body-start until the first input DMAs land
        # (~11-12.5us): an idle gap here would reset HAM ramp progress
        for _ in range(22):
            nc.tensor.matmul(wps, wz[:, :128], wz, start=True, stop=True)
        # force the GELU spline-table load (~2.7us) to happen now, during the
        # startup DMA shadow, instead of right before the first real GELU
        wact = const.tile([128, 1], f32)
        nc.scalar.activation(wact, wz[:, :1], GELU, bias=0.0)

        b1t = const.tile([128, 2, JL, NHC], f32)
        b2t = const.tile([128, 2, JL], f32)
        b1s = const.tile([2 * JL * NHC, 128], f32)
        b2s = const.tile([2 * JL, K], f32)

        def bias_dma():
            # biases: clean row-major staging DMA, then PE-transpose on chip.
            # Issued on the scalar queue (idle at startup) so sync/gpsimd can
            # issue the j=0 x and weight DMAs without delay.
            nc.scalar.dma_start(
                out=b1s, in_=b1.rearrange("c j (hc p) -> (c j hc) p", p=128)
            )
            nc.scalar.dma_start(out=b2s, in_=b2.rearrange("c j k -> (c j) k"))

        def bias1_stage():
            n1 = 2 * JL * NHC
            b1ps = ps2.tile([128, n1], f32, tag="p2")
            nc.tensor.transpose(b1ps, b1s, identity[:n1, :n1])
            nc.vector.tensor_copy(b1t.rearrange("p c j hc -> p (c j hc)"), b1ps)

        def bias2_stage():
            n2 = 2 * JL
            b2ps = ps2.tile([128, n2], f32, tag="p2")
            nc.tensor.transpose(b2ps, b2s, identity[:n2, :n2])
            nc.vector.tensor_copy(b2t.rearrange("p c j -> p (c j)"), b2ps)

        def load_j(j):
            # All j>=1 input DMAs go on the gpsimd queue, in need-order
            # (w1, x, w2), BEHIND j=0's critical set: the SDMA engines
            # round-robin between queues at packet granularity, so anything
            # issued concurrently with j=0's weights would steal half the
            # HBM bandwidth exactly when the PE is waiting for its first
            # operands. j=0's x rides the otherwise-empty sync queue in
            # parallel with its weights.
            w1t = wp.tile([128, 2, H], bf16, tag="w1t")  # [k, c, h]
            nc.gpsimd.dma_start(out=w1t, in_=w1[j])
            xq = nc.sync if j == 0 else nc.gpsimd
            # split real/imag: the first two L1 matmuls of a j only need xr
            xtr = xtp.tile([128, ROWS], bf16, tag="xtr")
            xq.dma_start(out=xtr, in_=xp[j, :, 0])
            xti = xtp.tile([128, ROWS], bf16, tag="xti")
            xq.dma_start(out=xti, in_=xp[j, :, 1])
            w2t = wp.tile([128, 2, NHC, K], bf16, tag="w2t")  # [p, c, hc, k']
            nc.gpsimd.dma_start(out=w2t, in_=w2[j])
            w1n = wnp.tile([128, H], bf16, tag="w1n")  # -w1[1]
            nc.vector.tensor_scalar_mul(w1n, w1t[:, 1], -1.0)
            w2n = wnp.tile([128, NHC, K], bf16, tag="w2n")  # -w2[1]
            nc.vector.tensor_scalar_mul(w2n, w2t[:, 1], -1.0)
            w2s = wnp.tile([128, NHC, K], bf16, tag="w2s")  # w2[0]+w2[1]
            nc.vector.tensor_add(w2s, w2t[:, 0], w2t[:, 1])
            return (w1t, w1n, w2t, w2n, w2s), (xtr, xti)

        def layer1(j, W, xt):
            w1t, w1n, w2t, w2n, w2s = W
            xtr, xti = xt
            o1r = o1p.tile([128, NHC, ROWS], bf16, tag="o1r")
            o1i = o1p.tile([128, NHC, ROWS], bf16, tag="o1i")
            for hc in range(NHC):
                hs = slice(hc * 128, (hc + 1) * 128)
                p1r = ps1.tile([128, ROWS], f32, tag="p1")
                p1i = ps1.tile([128, ROWS], f32, tag="p1")
                # xr-consuming matmuls first (xr lands before xi)
                nc.tensor.matmul(p1r, w1t[:, 0, hs], xtr, start=True, stop=False)
                nc.tensor.matmul(p1i, w1t[:, 1, hs], xtr, start=True, stop=False)
                nc.tensor.matmul(p1i, w1t[:, 0, hs], xti, start=False, stop=True)
                nc.tensor.matmul(p1r, w1n[:, hs], xti, start=False, stop=True)
                nc.scalar.activation(
                    o1i[:, hc], p1i, GELU, bias=b1t[:, 1, j, hc : hc + 1]
                )
                nc.scalar.activation(
                    o1r[:, hc], p1r, GELU, bias=b1t[:, 0, j, hc : hc + 1]
                )
            return o1r, o1i

        def layer2(j, W, o1r, o1i):
            w1t, w1n, w2t, w2n, w2s = W
            p2r = ps2.tile([128, ROWS], f32, tag="p2")
            p2i = ps2.tile([128, ROWS], f32, tag="p2")
            for hc in range(NHC):
                last = hc == NHC - 1
                if last:
                    # imag group stops first so its drain overlaps the final
                    # real matmuls (matters on the last j of the kernel)
                    nc.tensor.matmul(p2i, w2s[:, hc], o1i[:, hc], start=False, stop=True)
                    nc.tensor.matmul(p2r, w2t[:, 0, hc], o1r[:, hc], start=False, stop=False)
                    nc.tensor.matmul(p2r, w2n[:, hc], o1i[:, hc], start=False, stop=True)
                else:
                    nc.tensor.matmul(
                        p2r, w2t[:, 0, hc], o1r[:, hc], start=(hc == 0), stop=False
                    )
                    nc.tensor.matmul(p2r, w2n[:, hc], o1i[:, hc], start=False, stop=False)
                    nc.tensor.matmul(
                        p2i, w2s[:, hc], o1i[:, hc], start=(hc == 0), stop=False
                    )
            oti = outp.tile([128, ROWS], bf16, tag="ot")
            nc.vector.tensor_scalar_add(oti, p2i, b2t[:, 1, j : j + 1])
            nc.sync.dma_start(out=out[j, 1], in_=oti)
            otr = outp.tile([128, ROWS], bf16, tag="ot")
            nc.vector.tensor_scalar_add(otr, p2r, b2t[:, 0, j : j + 1])
            nc.sync.dma_start(out=out[j, 0], in_=otr)

        # software pipeline across j: PE order is L1(0), L1(1), L2(0),
        # L1(2), L2(1), ... so the PE is a full L1 block ahead of the GELUs
        # that L2 consumes.
        bias_dma()
        W, xt = load_j(0)
        # bias transposes run on the PE right after warmup, before the first
        # L1 matmuls (which are DMA-gated anyway), so b1t is ready well
        # before the first GELU
        bias1_stage()
        bias2_stage()
        o1 = layer1(0, W, xt)
        for j in range(JL):
            Wn = o1n = None
            if j + 1 < JL:
                Wn, xtn = load_j(j + 1)
                o1n = layer1(j + 1, Wn, xtn)
            layer2(j, W, *o1)
            W, o1 = Wn, o1n

    if not nc.is_finalized():
        nc.finalize()
    return nc


def _shard_inputs(x_real, x_imag, w1, b1, w2, b2):
    in_maps = []
    wcache = {}
    for jg in range(NJG):
        js = slice(jg * JL, (jg + 1) * JL)
        # weights identical across the 4 batch groups -- convert once
        w1h = np.ascontiguousarray(
            w1[:, js].transpose(1, 2, 0, 3)
        ).astype(BF16)  # [JL, K, 2, H]
        w2h = np.ascontiguousarray(
            w2[:, js].reshape(2, JL, NHC, 128, K).transpose(1, 3, 0, 2, 4)
        ).astype(BF16)  # [JL, p, 2, hc, k']
        b1h = np.ascontiguousarray(b1[:, js])
        b2h = np.ascontiguousarray(b2[:, js])
        wcache[jg] = (w1h, w2h, b1h, b2h)
        for rg in range(NRG):
            bs = slice(rg * BL, (rg + 1) * BL)
            # [BL, I, JL, K] -> [JL, K, BL*I]
            xr_s = x_real[bs, :, js, :].transpose(2, 3, 0, 1).reshape(JL, K, ROWS)
            xi_s = x_imag[bs, :, js, :].transpose(2, 3, 0, 1).reshape(JL, K, ROWS)
            xp = np.stack([xr_s, xi_s], axis=2).astype(BF16)  # [JL, K, 2, ROWS]
            in_maps.append(
                {
                    "xp": np.ascontiguousarray(xp),
                    "w1": w1h,
                    "w2": w2h,
                    "b1": b1h,
                    "b2": b2h,
                }
            )
    return in_maps


def _gather(results):
    out = np.empty((B, I, J, K), np.complex64)
    idx = 0
    for jg in range(NJG):
        for rg in range(NRG):
            js = slice(jg * JL, (jg + 1) * JL)
            bs = slice(rg * BL, (rg + 1) * BL)
            o = np.asarray(results[idx]["out"]).astype(np.float32)  # [13,2,128,512]
            oc = (o[:, 0] + 1j * o[:, 1]).astype(np.complex64)  # [13,128,512]
            # [j, k, rows] -> [rows, j, k] -> [BL, I, JL, K]
            out[bs, :, js, :] = oc.transpose(2, 0, 1).reshape(BL, I, JL, K)
            idx += 1
    return out


def run(trace=False, **inputs):
    from concourse.bass_utils import run_bass_kernel_spmd

    if "nc" not in _cache:
        _cache["nc"] = _build_nc()
    in_maps = _shard_inputs(
        np.asarray(inputs["x_real"], np.float32),
        np.asarray(inputs["x_imag"], np.float32),
        np.asarray(inputs["w1"], np.float32),
        np.asarray(inputs["b1"], np.float32),
        np.asarray(inputs["w2"], np.float32),
        np.asarray(inputs["b2"], np.float32),
    )
    res = run_bass_kernel_spmd(_cache["nc"], in_maps, list(range(8)), trace=trace)
    return _gather(res.results), res


def kernel(**inputs):
    out, _ = run(trace=False, **inputs)
    return out


# revision 22
# speedup vs baseline: 1.1938x; 1.1938x over previous
"""Trainium2 Bass kernel for nn_MlpMixer_18966575579742.

Complex-valued per-frequency (j) MLP:
  o1r = gelu(xr@w1[0] - xi@w1[1] + b1[0]);  o1i = gelu(xi@w1[0] + xr@w1[1] + b1[1])
  o2r = o1r@w2[0] - o1i@w2[1] + b2[0];      o2i = o1i@w2[0] + o1i@w2[1] + b2[1]
  (note: o2i intentionally uses o1i with BOTH w2[0] and w2[1], as in the source)

Sharding over 8 cores: 2 j-halves (13 each) x 4 batch-quarters (B=32 -> 512 rows).

Per-core dataflow, all matmuls in bf16 (1 PE pass @ ~216ns for N=512 vs fp32's
2 passes @ ~432ns -- 4x less PE time; fp32 PSUM accumulation keeps the
contraction exact, tolerance is 2e-2):
  - host pre-transposes x shards to [j, k, c, rows] bf16 so both real and
    imag arrive in one DMA per j, already in streaming layout
  - L1 uses the direct 4-matmul complex product accumulated in PSUM
    (p1r = xr@w1[0] + xi@(-w1[1]), p1i = xi@w1[0] + xr@w1[1]); vs Gauss
    3-mult this costs +1 bf16 pass per h-chunk but eliminates the 3 DVE
    combine ops per chunk that would otherwise make Vector the bottleneck
  - exact-erf GELU + per-partition b1 bias runs on ScalarE reading PSUM
    directly (ScalarE has the fast PSUM port), writing bf16 o1 to SBUF
  - L2 (w2 stationary, o1 moving): o2T [k'=128, rows] PSUM accumulated via
    w2[0], -w2[1] (real) and w2[0]+w2[1] (imag) -- 3 passes per h-chunk
  - DVE drains PSUM with fused per-partition b2 bias, writing bf16
  - output stays transposed [j, c, k', rows] bf16; host does the final
    transpose + complex interleave
  - biases are DMA'd in clean row-major staging tiles and PE-transposed once
  - DMA issue split across queues: x/out on sync, weights/bias on gpsimd
    (ScalarE issues no DMA -- it needs its full time for GELU)
  - software pipeline: L1(j+1) is issued to the PE before L2(j), so the PE
    never stalls waiting for GELU(j) to finish
"""

import sys

if "/opt/trn_rl_repo" not in sys.path:
    sys.path.insert(0, "/opt/trn_rl_repo")

import numpy as np
import ml_dtypes

BF16 = ml_dtypes.bfloat16

B, I, J, K, F = 128, 16, 26, 128, 4
H = K * F  # 512
NJG = 2  # j groups
NRG = 4  # row (batch) groups
JL = J // NJG  # 13 j per core
BL = B // NRG  # 32 batches per core
ROWS = BL * I  # 512 rows per core
NHC = H // 128  # 4 h-chunks

_cache = {}


def _build_nc():
    from contextlib import ExitStack

    import concourse.mybir as mybir
    import concourse.tile as tile
    from concourse import bacc
    from concourse.masks import make_identity

    f32 = mybir.dt.float32
    bf16 = mybir.dt.bfloat16
    nc = bacc.Bacc(None)

    # x arrives pre-transposed: [j, k, c, rows] (c = real/imag), bf16
    xp = nc.declare_dram_parameter("xp", [JL, K, 2, ROWS], bf16, isOutput=False)
    w1 = nc.declare_dram_parameter("w1", [JL, K, 2, H], bf16, isOutput=False)
    # w2 pre-shuffled to [j, p, c, hc, k'] where h = hc*128 + p
    w2 = nc.declare_dram_parameter("w2", [JL, K, 2, NHC, K], bf16, isOutput=False)
    b1 = nc.declare_dram_parameter("b1", [2, JL, H], f32, isOutput=False)
    b2 = nc.declare_dram_parameter("b2", [2, JL, K], f32, isOutput=False)
    # transposed output: [j, c, k', rows] bf16; host fixes layout
    out = nc.declare_dram_parameter("out", [JL, 2, K, ROWS], bf16, isOutput=True)

    GELU = mybir.ActivationFunctionType.Gelu

    with tile.TileContext(nc) as tc, ExitStack() as ctx:
        const = ctx.enter_context(tc.tile_pool(name="const", bufs=1))
        wp = ctx.enter_context(tc.tile_pool(name="wp", bufs=2))
        wnp = ctx.enter_context(tc.tile_pool(name="wnp", bufs=2))
        xtp = ctx.enter_context(tc.tile_pool(name="xtp", bufs=2))
        o1p = ctx.enter_context(tc.tile_pool(name="o1p", bufs=2))
        outp = ctx.enter_context(tc.tile_pool(name="outp", bufs=4))
        ps1 = ctx.enter_context(tc.tile_pool(name="ps1", bufs=4, space="PSUM"))
        ps2 = ctx.enter_context(tc.tile_pool(name="ps2", bufs=4, space="PSUM"))

        identity = const.tile([128, 128], f32)
        make_identity(nc, identity)

        # PE warm-up: the HAM clock gate keeps the PE at 1.2 GHz until it has
        # been busy for a full ~3.4us activity window. Burn that window on
        # dummy zero matmuls while the first DMAs are still in flight, so
        # every real matmul runs at 2.4 GHz. The memset goes on gpsimd, whose
        # const-init work runs during the NEFF preamble window.
        wz = const.tile([128, 256], bf16)
        nc.vector.memset(wz, 0.0)
        wps = ps1.tile([128, 256], f32, tag="p1")
        # enough to bridge from body-start until the first input DMAs land
        # (~11-12.5us): an idle gap here would reset HAM ramp progress
        for _ in range(22):
            nc.tensor.matmul(wps, wz[:, :128], wz, start=True, stop=True)
        # force the GELU spline-table load (~2.7us) to happen now, during the
        # startup DMA shadow, instead of right before the first real GELU
        wact = const.tile([128, 1], f32)
        nc.scalar.activation(wact, wz[:, :1], GELU, bias=0.0)

        b1t = const.tile([128, 2, JL, NHC], f32)
        b2t = const.tile([128, 2, JL], f32)
        b1s = const.tile([2 * JL * NHC, 128], f32)
        b2s = const.tile([2 * JL, K], f32)

        def bias_dma():
            # biases: clean row-major staging DMA, then PE-transpose on chip.
            # Issued on the scalar queue (idle at startup) so sync/gpsimd can
            # issue the j=0 x and weight DMAs without delay.
            nc.scalar.dma_start(
                out=b1s, in_=b1.rearrange("c j (hc p) -> (c j hc) p", p=128)
            )
            nc.scalar.dma_start(out=b2s, in_=b2.rearrange("c j k -> (c j) k"))

        def bias1_stage():
            n1 = 2 * JL * NHC
            b1ps = ps2.tile([128, n1], f32, tag="p2")
            nc.tensor.transpose(b1ps, b1s, identity[:n1, :n1])
            nc.vector.tensor_copy(b1t.rearrange("p c j hc -> p (c j hc)"), b1ps)

        def bias2_stage():
            n2 = 2 * JL
            b2ps = ps2.tile([128, n2], f32, tag="p2")
            nc.tensor.transpose(b2ps, b2s, identity[:n2, :n2])
            nc.vector.tensor_copy(b2t.rearrange("p c j -> p (c j)"), b2ps)

        def load_j(j):
            # All j>=1 input DMAs go on the gpsimd queue, in need-order
            # (w1, x, w2), BEHIND j=0's critical set: the SDMA engines
            # round-robin between queues at packet granularity, so anything
            # issued concurrently with j=0's weights would steal half the
            # HBM bandwidth exactly when the PE is waiting for its first
            # operands. j=0's x rides the otherwise-empty sync queue in
            # parallel with its weights.
            w1t = wp.tile([128, 2, H], bf16, tag="w1t")  # [k, c, h]
            nc.gpsimd.dma_start(out=w1t, in_=w1[j])
            xq = nc.sync if j == 0 else nc.gpsimd
            # split real/imag: the first two L1 matmuls of a j only need xr
            xtr = xtp.tile([128, ROWS], bf16, tag="xtr")
            xq.dma_start(out=xtr, in_=xp[j, :, 0])
            xti = xtp.tile([128, ROWS], bf16, tag="xti")
            xq.dma_start(out=xti, in_=xp[j, :, 1])
            w2t = wp.tile([128, 2, NHC, K], bf16, tag="w2t")  # [p, c, hc, k']
            nc.gpsimd.dma_start(out=w2t, in_=w2[j])
            w1n = wnp.tile([128, H], bf16, tag="w1n")  # -w1[1]
            nc.vector.tensor_scalar_mul(w1n, w1t[:, 1], -1.0)
            w2n = wnp.tile([128, NHC, K], bf16, tag="w2n")  # -w2[1]
            nc.vector.tensor_scalar_mul(w2n, w2t[:, 1], -1.0)
            w2s = wnp.tile([128, NHC, K], bf16, tag="w2s")  # w2[0]+w2[1]
            nc.vector.tensor_add(w2s, w2t[:, 0], w2t[:, 1])
            return (w1t, w1n, w2t, w2n, w2s), (xtr, xti)

        def layer1(j, W, xt):
            w1t, w1n, w2t, w2n, w2s = W
            xtr, xti = xt
            o1r = o1p.tile([128, NHC, ROWS], bf16, tag="o1r")
            o1i = o1p.tile([128, NHC, ROWS], bf16, tag="o1i")
            for hc in range(NHC):
                hs = slice(hc * 128, (hc + 1) * 128)
                p1r = ps1.tile([128, ROWS], f32, tag="p1")
                p1i = ps1.tile([128, ROWS], f32, tag="p1")
                # xr-consuming matmuls first (xr lands before xi)
                nc.tensor.matmul(p1r, w1t[:, 0, hs], xtr, start=True, stop=False)
                nc.tensor.matmul(p1i, w1t[:, 1, hs], xtr, start=True, stop=False)
                nc.tensor.matmul(p1i, w1t[:, 0, hs], xti, start=False, stop=True)
                nc.tensor.matmul(p1r, w1n[:, hs], xti, start=False, stop=True)
                nc.scalar.activation(
                    o1i[:, hc], p1i, GELU, bias=b1t[:, 1, j, hc : hc + 1]
                )
                nc.scalar.activation(
                    o1r[:, hc], p1r, GELU, bias=b1t[:, 0, j, hc : hc + 1]
                )
            return o1r, o1i

        def layer2(j, W, o1r, o1i):
            w1t, w1n, w2t, w2n, w2s = W
            p2r = ps2.tile([128, ROWS], f32, tag="p2")
            p2i = ps2.tile([128, ROWS], f32, tag="p2")
            for hc in range(NHC):
                last = hc == NHC - 1
                if last:
                    # imag group stops first so its drain overlaps the final
                    # real matmuls (matters on the last j of the kernel)
                    nc.tensor.matmul(p2i, w2s[:, hc], o1i[:, hc], start=False, stop=True)
                    nc.tensor.matmul(p2r, w2t[:, 0, hc], o1r[:, hc], start=False, stop=False)
                    nc.tensor.matmul(p2r, w2n[:, hc], o1i[:, hc], start=False, stop=True)
                else:
                    nc.tensor.matmul(
                        p2r, w2t[:, 0, hc], o1r[:, hc], start=(hc == 0), stop=False
                    )
                    nc.tensor.matmul(p2r, w2n[:, hc], o1i[:, hc], start=False, stop=False)
                    nc.tensor.matmul(
                        p2i, w2s[:, hc], o1i[:, hc], start=(hc == 0), stop=False
                    )
            oti = outp.tile([128, ROWS], bf16, tag="ot")
            nc.vector.tensor_scalar_add(oti, p2i, b2t[:, 1, j : j + 1])
            nc.sync.dma_start(out=out[j, 1], in_=oti)
            otr = outp.tile([128, ROWS], bf16, tag="ot")
            nc.vector.tensor_scalar_add(otr, p2r, b2t[:, 0, j : j + 1])
            # last j: the final DMA issues on the (by now idle) scalar queue
            # so it doesn't serialize behind oti's issue on sync
            oq = nc.scalar if j == JL - 1 else nc.sync
            oq.dma_start(out=out[j, 0], in_=otr)

        # software pipeline across j: PE order is L1(0), L1(1), L2(0),
        # L1(2), L2(1), ... so the PE is a full L1 block ahead of the GELUs
        # that L2 consumes.
        bias_dma()
        W, xt = load_j(0)
        # bias transposes run on the PE right after warmup, before the first
        # L1 matmuls (which are DMA-gated anyway), so b1t is ready well
        # before the first GELU
        bias1_stage()
        bias2_stage()
        o1 = layer1(0, W, xt)
        for j in range(JL):
            Wn = o1n = None
            if j + 1 < JL:
                Wn, xtn = load_j(j + 1)
                o1n = layer1(j + 1, Wn, xtn)
            layer2(j, W, *o1)
            W, o1 = Wn, o1n

    if not nc.is_finalized():
        nc.finalize()
    return nc


def _shard_inputs(x_real, x_imag, w1, b1, w2, b2):
    in_maps = []
    wcache = {}
    for jg in range(NJG):
        js = slice(jg * JL, (jg + 1) * JL)
        # weights identical across the 4 batch groups -- convert once
        w1h = np.ascontiguousarray(
            w1[:, js].transpose(1, 2, 0, 3)
        ).astype(BF16)  # [JL, K, 2, H]
        w2h = np.ascontiguousarray(
            w2[:, js].reshape(2, JL, NHC, 128, K).transpose(1, 3, 0, 2, 4)
        ).astype(BF16)  # [JL, p, 2, hc, k']
        b1h = np.ascontiguousarray(b1[:, js])
        b2h = np.ascontiguousarray(b2[:, js])
        wcache[jg] = (w1h, w2h, b1h, b2h)
        for rg in range(NRG):
            bs = slice(rg * BL, (rg + 1) * BL)
            # [BL, I, JL, K] -> [JL, K, BL*I]
            xr_s = x_real[bs, :, js, :].transpose(2, 3, 0, 1).reshape(JL, K, ROWS)
            xi_s = x_imag[bs, :, js, :].transpose(2, 3, 0, 1).reshape(JL, K, ROWS)
            xp = np.stack([xr_s, xi_s], axis=2).astype(BF16)  # [JL, K, 2, ROWS]
            in_maps.append(
                {
                    "xp": np.ascontiguousarray(xp),
                    "w1": w1h,
                    "w2": w2h,
                    "b1": b1h,
                    "b2": b2h,
                }
            )
    return in_maps


def _gather(results):
    out = np.empty((B, I, J, K), np.complex64)
    idx = 0
    for jg in range(NJG):
        for rg in range(NRG):
            js = slice(jg * JL, (jg + 1) * JL)
            bs = slice(rg * BL, (rg + 1) * BL)
            o = np.asarray(results[idx]["out"]).astype(np.float32)  # [13,2,128,512]
            oc = (o[:, 0] + 1j * o[:, 1]).astype(np.complex64)  # [13,128,512]
            # [j, k, rows] -> [rows, j, k] -> [BL, I, JL, K]
            out[bs, :, js, :] = oc.transpose(2, 0, 1).reshape(BL, I, JL, K)
            idx += 1
    return out


def run(trace=False, **inputs):
    from concourse.bass_utils import run_bass_kernel_spmd

    if "nc" not in _cache:
        _cache["nc"] = _build_nc()
    in_maps = _shard_inputs(
        np.asarray(inputs["x_real"], np.float32),
        np.asarray(inputs["x_imag"], np.float32),
        np.asarray(inputs["w1"], np.float32),
        np.asarray(inputs["b1"], np.float32),
        np.asarray(inputs["w2"], np.float32),
        np.asarray(inputs["b2"], np.float32),
    )
    res = run_bass_kernel_spmd(_cache["nc"], in_maps, list(range(8)), trace=trace)
    return _gather(res.results), res


def kernel(**inputs):
    out, _ = run(trace=False, **inputs)
    return out


# revision 27
# speedup vs baseline: 1.3121x; 1.0991x over previous
"""Trainium2 Bass kernel for nn_MlpMixer_18966575579742.

Complex-valued per-frequency (j) MLP:
  o1r = gelu(xr@w1[0] - xi@w1[1] + b1[0]);  o1i = gelu(xi@w1[0] + xr@w1[1] + b1[1])
  o2r = o1r@w2[0] - o1i@w2[1] + b2[0];      o2i = o1i@w2[0] + o1i@w2[1] + b2[1]
  (note: o2i intentionally uses o1i with BOTH w2[0] and w2[1], as in the source)

Sharding over 8 cores: 2 j-halves (13 each) x 4 batch-quarters (B=32 -> 512 rows).

Per-core dataflow, all matmuls in bf16 (1 PE pass @ ~216ns for N=512 vs fp32's
2 passes @ ~432ns -- 4x less PE time; fp32 PSUM accumulation keeps the
contraction exact, tolerance is 2e-2):
  - host pre-transposes x shards to [j, k, c, rows] bf16 so both real and
    imag arrive in one DMA per j, already in streaming layout
  - L1 uses the direct 4-matmul complex product accumulated in PSUM
    (p1r = xr@w1[0] + xi@(-w1[1]), p1i = xi@w1[0] + xr@w1[1]); vs Gauss
    3-mult this costs +1 bf16 pass per h-chunk but eliminates the 3 DVE
    combine ops per chunk that would otherwise make Vector the bottleneck
  - exact-erf GELU + per-partition b1 bias runs on ScalarE reading PSUM
    directly (ScalarE has the fast PSUM port), writing bf16 o1 to SBUF
  - L2 (w2 stationary, o1 moving): o2T [k'=128, rows] PSUM accumulated via
    w2[0], -w2[1] (real) and w2[0]+w2[1] (imag) -- 3 passes per h-chunk
  - DVE drains PSUM with fused per-partition b2 bias, writing bf16
  - output stays transposed [j, c, k', rows] bf16; host does the final
    transpose + complex interleave
  - biases are DMA'd in clean row-major staging tiles and PE-transposed once
  - DMA issue split across queues: x/out on sync, weights/bias on gpsimd
    (ScalarE issues no DMA -- it needs its full time for GELU)
  - software pipeline: L1(j+1) is issued to the PE before L2(j), so the PE
    never stalls waiting for GELU(j) to finish
"""

import sys

if "/opt/trn_rl_repo" not in sys.path:
    sys.path.insert(0, "/opt/trn_rl_repo")

import numpy as np
import ml_dtypes

BF16 = ml_dtypes.bfloat16

B, I, J, K, F = 128, 16, 26, 128, 4
H = K * F  # 512
NJG = 2  # j groups
NRG = 4  # row (batch) groups
JL = J // NJG  # 13 j per core
BL = B // NRG  # 32 batches per core
ROWS = BL * I  # 512 rows per core
NHC = H // 128  # 4 h-chunks

_cache = {}


def _build_nc():
    from contextlib import ExitStack

    import concourse.mybir as mybir
    import concourse.tile as tile
    from concourse import bacc
    from concourse.masks import make_identity

    f32 = mybir.dt.float32
    bf16 = mybir.dt.bfloat16
    nc = bacc.Bacc(None)

    # x arrives pre-transposed: [j, k, c, rows] (c = real/imag), bf16
    xp = nc.declare_dram_parameter("xp", [JL, K, 2, ROWS], bf16, isOutput=False)
    w1 = nc.declare_dram_parameter("w1", [JL, K, 2, H], bf16, isOutput=False)
    # w2 pre-shuffled to [j, p, c, hc, k'] where h = hc*128 + p
    w2 = nc.declare_dram_parameter("w2", [JL, K, 2, NHC, K], bf16, isOutput=False)
    b1 = nc.declare_dram_parameter("b1", [2, JL, H], f32, isOutput=False)
    b2 = nc.declare_dram_parameter("b2", [2, JL, K], f32, isOutput=False)
    # transposed output: [j, c, k', rows] bf16; host fixes layout
    out = nc.declare_dram_parameter("out", [JL, 2, K, ROWS], bf16, isOutput=True)

    GELU = mybir.ActivationFunctionType.Gelu

    with tile.TileContext(nc) as tc, ExitStack() as ctx:
        const = ctx.enter_context(tc.tile_pool(name="const", bufs=1))
        wp = ctx.enter_context(tc.tile_pool(name="wp", bufs=2))
        wnp = ctx.enter_context(tc.tile_pool(name="wnp", bufs=2))
        xtp = ctx.enter_context(tc.tile_pool(name="xtp", bufs=2))
        o1p = ctx.enter_context(tc.tile_pool(name="o1p", bufs=2))
        outp = ctx.enter_context(tc.tile_pool(name="outp", bufs=4))
        ps1 = ctx.enter_context(tc.tile_pool(name="ps1", bufs=4, space="PSUM"))
        ps2 = ctx.enter_context(tc.tile_pool(name="ps2", bufs=4, space="PSUM"))

        identity = const.tile([128, 128], f32)
        make_identity(nc, identity)

        # PE warm-up: the HAM clock gate keeps the PE at 1.2 GHz until it has
        # been busy for a full ~3.4us activity window. Burn that window on
        # dummy zero matmuls while the first DMAs are still in flight, so
        # every real matmul runs at 2.4 GHz. The memset goes on gpsimd, whose
        # const-init work runs during the NEFF preamble window.
        wz = const.tile([128, 256], bf16)
        nc.vector.memset(wz, 0.0)
        wps = ps1.tile([128, 256], f32, tag="p1")
        # enough to bridge from body-start until the first input DMAs land
        # (~11-12.5us): an idle gap here would reset HAM ramp progress
        for _ in range(22):
            nc.tensor.matmul(wps, wz[:, :128], wz, start=True, stop=True)
        # force the GELU spline-table load (~2.7us) to happen now, during the
        # startup DMA shadow, instead of right before the first real GELU
        wact = const.tile([128, 1], f32)
        nc.scalar.activation(wact, wz[:, :1], GELU, bias=0.0)

        b1t = const.tile([128, 2, JL, NHC], f32)
        b2t = const.tile([128, 2, JL], f32)
        b2x = const.tile([128, JL], f32)  # b2[0] + b2[1], per-partition k'
        b1s = const.tile([2 * JL * NHC, 128], f32)
        b2s = const.tile([2 * JL, K], f32)

        def bias_dma():
            # biases: clean row-major staging DMA, then PE-transpose on chip.
            # Issued on the scalar queue (idle at startup) so sync/gpsimd can
            # issue the j=0 x and weight DMAs without delay.
            nc.scalar.dma_start(
                out=b1s, in_=b1.rearrange("c j (hc p) -> (c j hc) p", p=128)
            )
            nc.scalar.dma_start(out=b2s, in_=b2.rearrange("c j k -> (c j) k"))

        def bias1_stage():
            n1 = 2 * JL * NHC
            b1ps = ps2.tile([128, n1], f32, tag="p2")
            nc.tensor.transpose(b1ps, b1s, identity[:n1, :n1])
            nc.vector.tensor_copy(b1t.rearrange("p c j hc -> p (c j hc)"), b1ps)

        def bias2_stage():
            n2 = 2 * JL
            b2ps = ps2.tile([128, n2], f32, tag="p2")
            nc.tensor.transpose(b2ps, b2s, identity[:n2, :n2])
            nc.vector.tensor_copy(b2t.rearrange("p c j -> p (c j)"), b2ps)
            nc.vector.tensor_add(b2x, b2t[:, 0], b2t[:, 1])

        def load_j(j):
            # All j>=1 input DMAs go on the gpsimd queue, in need-order
            # (w1, x, w2), BEHIND j=0's critical set: the SDMA engines
            # round-robin between queues at packet granularity, so anything
            # issued concurrently with j=0's weights would steal half the
            # HBM bandwidth exactly when the PE is waiting for its first
            # operands. j=0's x rides the otherwise-empty sync queue in
            # parallel with its weights.
            w1t = wp.tile([128, 2, H], bf16, tag="w1t")  # [k, c, h]
            nc.gpsimd.dma_start(out=w1t, in_=w1[j])
            xq = nc.sync if j == 0 else nc.gpsimd
            # split real/imag: the first two L1 matmuls of a j only need xr
            xtr = xtp.tile([128, ROWS], bf16, tag="xtr")
            xq.dma_start(out=xtr, in_=xp[j, :, 0])
            xti = xtp.tile([128, ROWS], bf16, tag="xti")
            xq.dma_start(out=xti, in_=xp[j, :, 1])
            w2t = wp.tile([128, 2, NHC, K], bf16, tag="w2t")  # [p, c, hc, k']
            nc.gpsimd.dma_start(out=w2t, in_=w2[j])
            w1n = wnp.tile([128, H], bf16, tag="w1n")  # -w1[1]
            nc.vector.tensor_scalar_mul(w1n, w1t[:, 1], -1.0)
            w2s = wnp.tile([128, NHC, K], bf16, tag="w2s")  # w2[0]+w2[1]
            nc.vector.tensor_add(w2s, w2t[:, 0], w2t[:, 1])
            return (w1t, w1n, w2t, w2s), (xtr, xti)

        def layer1(j, W, xt):
            w1t, w1n, w2t, w2s = W
            xtr, xti = xt
            o1r = o1p.tile([128, NHC, ROWS], bf16, tag="o1r")
            o1i = o1p.tile([128, NHC, ROWS], bf16, tag="o1i")
            o1s = o1p.tile([128, NHC, ROWS], bf16, tag="o1s")
            for hc in range(NHC):
                hs = slice(hc * 128, (hc + 1) * 128)
                p1r = ps1.tile([128, ROWS], f32, tag="p1")
                p1i = ps1.tile([128, ROWS], f32, tag="p1")
                # xr-consuming matmuls first (xr lands before xi)
                nc.tensor.matmul(p1r, w1t[:, 0, hs], xtr, start=True, stop=False)
                nc.tensor.matmul(p1i, w1t[:, 1, hs], xtr, start=True, stop=False)
                nc.tensor.matmul(p1i, w1t[:, 0, hs], xti, start=False, stop=True)
                nc.tensor.matmul(p1r, w1n[:, hs], xti, start=False, stop=True)
                nc.scalar.activation(
                    o1i[:, hc], p1i, GELU, bias=b1t[:, 1, j, hc : hc + 1]
                )
                nc.scalar.activation(
                    o1r[:, hc], p1r, GELU, bias=b1t[:, 0, j, hc : hc + 1]
                )
                # s = o1r + o1i feeds L2's m1 matmul (bf16 SBUF add, 2x rate)
                nc.vector.tensor_add(o1s[:, hc], o1r[:, hc], o1i[:, hc])
            return o1r, o1i, o1s

        def layer2(j, W, o1r, o1i, o1s):
            # Algebraic 2-matmul L2 (exploits o2i reusing o1i with BOTH w2
            # halves, as the reference does):
            #   m2 = o1i@(w2[0]+w2[1])          -> o2i = m2 + b2i
            #   m1 = (o1r+o1i)@w2[0]            -> o2r = m1 - m2 + b2r
            # since m1 - m2 = o1r@w2[0] - o1i@w2[1]. The subtraction runs on
            # DVE (which has slack) instead of a third PE pass per chunk:
            #   otr = (m1 - oti) + (b2r + b2i)  [oti = m2 + b2i already]
            w1t, w1n, w2t, w2s = W
            m2 = ps2.tile([128, ROWS], f32, tag="p2")
            m1 = ps2.tile([128, ROWS], f32, tag="p2")
            for hc in range(NHC):
                last = hc == NHC - 1
                nc.tensor.matmul(
                    m2, w2s[:, hc], o1i[:, hc], start=(hc == 0), stop=last
                )
                nc.tensor.matmul(
                    m1, w2t[:, 0, hc], o1s[:, hc], start=(hc == 0), stop=last
                )
            oti = outp.tile([128, ROWS], bf16, tag="ot")
            nc.vector.tensor_scalar_add(oti, m2, b2t[:, 1, j : j + 1])
            nc.sync.dma_start(out=out[j, 1], in_=oti)
            ot0 = outp.tile([128, ROWS], bf16, tag="ot0")
            nc.vector.tensor_sub(ot0, m1, oti)
            otr = outp.tile([128, ROWS], bf16, tag="ot")
            nc.vector.tensor_scalar_add(otr, ot0, b2x[:, j : j + 1])
            # last j: the final DMA issues on the (by now idle) scalar queue
            # so it doesn't serialize behind oti's issue on sync
            oq = nc.scalar if j == JL - 1 else nc.sync
            oq.dma_start(out=out[j, 0], in_=otr)

        # software pipeline across j: PE order is L1(0), L1(1), L2(0),
        # L1(2), L2(1), ... so the PE is a full L1 block ahead of the GELUs
        # that L2 consumes.
        bias_dma()
        W, xt = load_j(0)
        # bias transposes run on the PE right after warmup, before the first
        # L1 matmuls (which are DMA-gated anyway), so b1t is ready well
        # before the first GELU
        bias1_stage()
        bias2_stage()
        o1 = layer1(0, W, xt)
        for j in range(JL):
            Wn = o1n = None
            if j + 1 < JL:
                Wn, xtn = load_j(j + 1)
                o1n = layer1(j + 1, Wn, xtn)
            layer2(j, W, *o1)
            W, o1 = Wn, o1n

    if not nc.is_finalized():
        nc.finalize()
    return nc


def _shard_inputs(x_real, x_imag, w1, b1, w2, b2):
    in_maps = []
    wcache = {}
    for jg in range(NJG):
        js = slice(jg * JL, (jg + 1) * JL)
        # weights identical across the 4 batch groups -- convert once
        w1h = np.ascontiguousarray(
            w1[:, js].transpose(1, 2, 0, 3)
        ).astype(BF16)  # [JL, K, 2, H]
        w2h = np.ascontiguousarray(
            w2[:, js].reshape(2, JL, NHC, 128, K).transpose(1, 3, 0, 2, 4)
        ).astype(BF16)  # [JL, p, 2, hc, k']
        b1h = np.ascontiguousarray(b1[:, js])
        b2h = np.ascontiguousarray(b2[:, js])
        wcache[jg] = (w1h, w2h, b1h, b2h)
        for rg in range(NRG):
            bs = slice(rg * BL, (rg + 1) * BL)
            # [BL, I, JL, K] -> [JL, K, BL*I]
            xr_s = x_real[bs, :, js, :].transpose(2, 3, 0, 1).reshape(JL, K, ROWS)
            xi_s = x_imag[bs, :, js, :].transpose(2, 3, 0, 1).reshape(JL, K, ROWS)
            xp = np.stack([xr_s, xi_s], axis=2).astype(BF16)  # [JL, K, 2, ROWS]
            in_maps.append(
                {
                    "xp": np.ascontiguousarray(xp),
                    "w1": w1h,
                    "w2": w2h,
                    "b1": b1h,
                    "b2": b2h,
                }
            )
    return in_maps


def _gather(results):
    out = np.empty((B, I, J, K), np.complex64)
    idx = 0
    for jg in range(NJG):
        for rg in range(NRG):
            js = slice(jg * JL, (jg + 1) * JL)
            bs = slice(rg * BL, (rg + 1) * BL)
            o = np.asarray(results[idx]["out"]).astype(np.float32)  # [13,2,128,512]
            oc = (o[:, 0] + 1j * o[:, 1]).astype(np.complex64)  # [13,128,512]
            # [j, k, rows] -> [rows, j, k] -> [BL, I, JL, K]
            out[bs, :, js, :] = oc.transpose(2, 0, 1).reshape(BL, I, JL, K)
            idx += 1
    return out


def run(trace=False, **inputs):
    from concourse.bass_utils import run_bass_kernel_spmd

    if "nc" not in _cache:
        _cache["nc"] = _build_nc()
    in_maps = _shard_inputs(
        np.asarray(inputs["x_real"], np.float32),
        np.asarray(inputs["x_imag"], np.float32),
        np.asarray(inputs["w1"], np.float32),
        np.asarray(inputs["b1"], np.float32),
        np.asarray(inputs["w2"], np.float32),
        np.asarray(inputs["b2"], np.float32),
    )
    res = run_bass_kernel_spmd(_cache["nc"], in_maps, list(range(8)), trace=trace)
    return _gather(res.results), res


def kernel(**inputs):
    out, _ = run(trace=False, **inputs)
    return out
